# revision 1
# baseline (speedup 1.0000x reference)
"""Trainium2 Bass kernel for nn_CIAM patch-attention module.

Shapes (hardcoded): x [8, 64, 256, 256] f32, size=4.
Sharding: pure data parallel - one sample per NeuronCore (8 cores).

Per-core structure: the image is split into TOP/BOTTOM halves (128 rows each)
processed as two independent pipelines (patches never cross the boundary), so
DMA and compute overlap across halves. Within a half: partition p = image row,
free dim = c*256 + w (w = wi*4 + b). All channel/b reductions are free-axis
DVE ops (bf16, 2x mode); the patch-row (a) folds ride the PE transposes used
for the 64x64 FC (fold over free columns after transposing); sigmoid +
duplication/expansion run on ACT; loads/stores are 1KB-run SWDGE cast DMAs.
"""
import sys
sys.path.insert(0, "/opt/trn_rl_repo")
import numpy as np

_CACHE = {}

B, C, H, W = 8, 64, 256, 256
S = 4
P = 128                # partitions = rows of one half-image
NV = 2                 # image halves (top/bottom)
HIV = P // S           # 32 patch rows per half
WI = W // S            # 64 patch cols
FPC = W                # free elems per channel (one row)
FH = C * FPC           # 16384 free elems per partition per half
CT = 8                 # channels per load tile
NT = C // CT           # 8 tiles
TF = CT * FPC          # 2048 free elems per (half, tile)


def _build():
    import concourse.bass as bass
    import concourse.bacc as bacc
    import concourse.tile as tile
    from concourse import mybir
    from concourse.masks import make_identity

    f32 = mybir.dt.float32
    bf16 = mybir.dt.bfloat16
    AL = mybir.AluOpType
    AF = mybir.ActivationFunctionType

    nc = bacc.Bacc("TRN2", target_bir_lowering=False, debug=False, num_devices=8)

    x_d = nc.dram_tensor("x", [C, H, W], f32, kind="ExternalInput")
    fcwT_d = nc.dram_tensor("fcwT", [C, C], bf16, kind="ExternalInput")
    fcb_d = nc.dram_tensor("fcb", [C], f32, kind="ExternalInput")
    cws_d = nc.dram_tensor("cws", [6], f32, kind="ExternalInput")
    emat_d = nc.dram_tensor("emat", [C, C * S], bf16, kind="ExternalInput")
    y_d = nc.dram_tensor("y", [C, H, W], f32, kind="ExternalOutput")

    # DRAM views: [half, row-in-half, c, w]
    x_v = x_d[:].rearrange("c (v r) w -> v r c w", v=NV)
    y_v = y_d[:].rearrange("c (v r) w -> v r c w", v=NV)

    with tile.TileContext(nc) as tc:
        with tc.tile_pool(name="big", bufs=1) as big, \
             tc.tile_pool(name="med", bufs=2) as med, \
             tc.tile_pool(name="sm", bufs=2) as sm, \
             tc.tile_pool(name="consts", bufs=1) as consts, \
             tc.tile_pool(name="ps", bufs=1, space="PSUM") as ps:

            # ---- constants ----
            fcw = consts.tile([C, C], bf16)
            nc.sync.dma_start(out=fcw, in_=fcwT_d[:])             # pre-cast bf16, HWDGE
            fcb = consts.tile([C, 1], f32)
            nc.sync.dma_start(out=fcb, in_=fcb_d[:].unsqueeze(1))
            cws = consts.tile([P, 6], f32)
            nc.sync.dma_start(out=cws, in_=bass.AP(tensor=cws_d, offset=0, ap=[[0, P], [1, 6]]))
            emat = consts.tile([C, C * S], bf16)
            nc.sync.dma_start(out=emat, in_=emat_d[:])
            ident = consts.tile([P, P], bf16)
            make_identity(nc, ident)
            identf = consts.tile([P, P], f32)
            make_identity(nc, identf)

            def emit_half(v):
                yield
                # ---------- Phase 1: load + max over b (in-row patch pixels) ----
                xbs = []   # (tile, first-ct, n-ct)
                chmaxB = med.tile([P, C * WI], bf16, tag="chmax", bufs=2)  # wi-major: wi*64+c
                sizes = [1] * NT
                ct0 = 0
                for nct in sizes:
                    xt = big.tile([P, nct * TF], bf16, tag=f"xb{v}", bufs=NT)
                    xbs.append((xt, ct0, nct))
                    nc.gpsimd.dma_start(out=xt.rearrange("p (c w) -> p c w", c=nct * CT),
                                        in_=x_v[v, :, ct0 * CT:(ct0 + nct) * CT, :])
                    for s_ in range(nct):
                        ct = ct0 + s_
                        v4 = xt[:, s_ * TF:(s_ + 1) * TF].rearrange("p (r pr u) -> p r pr u", pr=2, u=2)
                        r1 = sm.tile([P, CT * WI, 2], bf16, tag="r1", bufs=1)
                        nc.vector.tensor_tensor(out=r1, in0=v4[:, :, 0, :], in1=v4[:, :, 1, :], op=AL.max)
                        outv = chmaxB.rearrange("p (wi c) -> p c wi", c=C)[:, ct * CT:(ct + 1) * CT, :]
                        nc.vector.tensor_tensor(out=outv, in0=r1[:, :, 0], in1=r1[:, :, 1], op=AL.max)
                    ct0 += nct

                yield
                # ---------- Phase 2: FC attention -> m_e ------------------------
                # per group of 8 wi: build rhs [c, 8*32], one fc matmul (N=256),
                # one batched sigmoid (+a-dup), 8 transpose+b-expand matmuls with
                # the constant E matrix, one batched evacuation into m_e.
                # m_e as 4 wi-quarter tiles [c, wl(16), b] so P3 can start per quarter
                m_eqs = []
                for q_ in range(4):
                    m_eq = med.tile([P, C * W // 4], bf16, tag="me", bufs=4)
                    m_eqs.append(m_eq)
                GW = 8                       # wi per group
                for g in range(WI // GW):
                    # 4 transposed chmax slices into one psum tile, one evac,
                    # one batched a-fold, two fold+scatter ops -> rhs_w
                    pa4 = ps.tile([P, 4 * P], bf16, tag="pa", bufs=2)
                    for j2 in range(4):
                        j = g * 4 + j2
                        nc.tensor.transpose(pa4[:, j2 * P:(j2 + 1) * P],
                                            chmaxB[:, j * P:(j + 1) * P], ident)
                    pae4 = sm.tile([P, 4 * P], bf16, tag="pae", bufs=1)
                    nc.scalar.copy(out=pae4, in_=pa4)
                    pav = pae4.rearrange("q (jj hi a) -> q (jj hi) a", jj=4, a=S)
                    f1 = sm.tile([P, 4 * HIV, 2], bf16, tag="f1", bufs=1)
                    nc.vector.tensor_tensor(out=f1, in0=pav[:, :, 0:2], in1=pav[:, :, 2:4], op=AL.max)
                    rhs_w = sm.tile([C, GW * HIV], bf16, tag="rhs_w", bufs=2)
                    rhs_b = rhs_w.rearrange("c (blk hi) -> c blk hi", hi=HIV)
                    for k in range(2):
                        # block index (2*jj + k) maps to wi = g*8 + block
                        nc.vector.tensor_tensor(
                            out=rhs_b[:, k:GW:2, :],
                            in0=f1[k * C:(k + 1) * C, :, 0].rearrange("c (jj hi) -> c jj hi", jj=4),
                            in1=f1[k * C:(k + 1) * C, :, 1].rearrange("c (jj hi) -> c jj hi", jj=4),
                            op=AL.max)
                    pmw = ps.tile([C, GW * HIV], f32, tag="pmw", bufs=2)
                    nc.tensor.matmul(pmw, fcw, rhs_w, start=True, stop=True)
                    # sigmoid + duplicate each hi column over the 4 patch rows
                    s2w = sm.tile([C, GW * P], bf16, tag="s2w", bufs=1)
                    nc.scalar.activation(
                        out=s2w.rearrange("c (wl hi a) -> c wl hi a", wl=GW, a=S),
                        in_=pmw.rearrange("c (wl hi) -> c wl hi", wl=GW).unsqueeze(3).broadcast_to([C, GW, HIV, S]),
                        func=AF.Sigmoid, bias=fcb, scale=1.0)
                    for sg in range(2):
                        pe4 = ps.tile([P, GW // 2 * C * S], f32, tag="pe4", bufs=1)
                        for wl2 in range(GW // 2):
                            wl = sg * (GW // 2) + wl2
                            nc.tensor.matmul(pe4[:, wl2 * C * S:(wl2 + 1) * C * S],
                                             s2w[:, wl * P:(wl + 1) * P], emat,
                                             start=True, stop=True)
                        # batched evacuation: psum [(wl c b)] -> m_eq [c*64 + wl*4 + b]
                        w0l = (g % 2) * GW + sg * (GW // 2)
                        me_v = m_eqs[g // 2].rearrange("p (c wi b) -> p wi c b", c=C, b=S)[:, w0l:w0l + GW // 2, :, :]
                        nc.scalar.copy(out=me_v, in_=pe4.rearrange("p (wl c b) -> p wl c b", wl=GW // 2, b=S))

                yield
                # ---------- Phase 3: p1 = x * m ---------------------------------
                p1s = []
                for xt, ct0, nct in xbs:
                    p1t = big.tile([P, nct * TF], bf16, tag="p1", bufs=2 * len(xbs))
                    p1s.append((p1t, ct0, nct))
                    ncc = nct * CT
                    for q_ in range(4):
                        WQ = W // 4
                        nc.vector.tensor_tensor(
                            out=p1t.rearrange("p (c w) -> p c w", c=ncc)[:, :, q_ * WQ:(q_ + 1) * WQ],
                            in0=xt.rearrange("p (c w) -> p c w", c=ncc)[:, :, q_ * WQ:(q_ + 1) * WQ],
                            in1=m_eqs[q_].rearrange("p (c wb) -> p c wb", c=C)[:, ct0 * CT:(ct0 + nct) * CT, :],
                            op=AL.mult)

                yield
                # ---------- Phase 4: channel stats + gates ----------------------
                st = big.tile([P, FH // 2], bf16, tag="tree", bufs=1)
                nh = len(p1s) // 2
                for q_ in range(nh):
                    qo = q_ * (FH // 2 // nh)
                    nc.vector.tensor_tensor(out=st[:, qo:qo + FH // 2 // nh],
                                            in0=p1s[q_][0], in1=p1s[q_ + nh][0], op=AL.add)
                n = FH // 4
                while n >= FPC * 2:
                    nc.vector.tensor_tensor(out=st[:, :n], in0=st[:, :n], in1=st[:, n:2 * n], op=AL.add)
                    n //= 2
                s_raw = sm.tile([P, FPC], f32, tag="s_raw", bufs=1)
                nc.vector.tensor_tensor(out=s_raw, in0=st[:, :FPC], in1=st[:, FPC:2 * FPC], op=AL.add)

                mt = big.tile([P, FH // 2], bf16, tag="tree", bufs=1)
                for q_ in range(nh):
                    qo = q_ * (FH // 2 // nh)
                    nc.vector.tensor_tensor(out=mt[:, qo:qo + FH // 2 // nh],
                                            in0=p1s[q_][0], in1=p1s[q_ + nh][0], op=AL.max)
                n = FH // 4
                while n >= FPC * 2:
                    nc.vector.tensor_tensor(out=mt[:, :n], in0=mt[:, :n], in1=mt[:, n:2 * n], op=AL.max)
                    n //= 2
                mx = sm.tile([P, FPC], bf16, tag="mx", bufs=1)
                nc.vector.tensor_tensor(out=mx, in0=mt[:, :FPC], in1=mt[:, FPC:2 * FPC], op=AL.max)

                # g1 = sigmoid(cw0 * s_raw/64 + cw1 * mx + cb)
                t1 = sm.tile([P, FPC], bf16, tag="t1", bufs=1)
                nc.vector.tensor_scalar(out=t1, in0=s_raw, scalar1=cws[:, 0:1], scalar2=1.0 / C,
                                        op0=AL.mult, op1=AL.mult)
                t2 = sm.tile([P, FPC], f32, tag="t2", bufs=1)
                nc.vector.tensor_scalar_mul(out=t2, in0=mx, scalar1=cws[:, 1:2])
                nc.vector.tensor_tensor(out=t1, in0=t1, in1=t2, op=AL.add)
                g1 = sm.tile([P, FPC], f32, tag="g1", bufs=1)
                nc.scalar.activation(out=g1, in_=t1, func=AF.Sigmoid, bias=cws[:, 2:3], scale=1.0)

                # per-patch partial stats over b (per row): then fold a after transpose
                u = sm.tile([P, FPC], f32, tag="t2", bufs=1)
                nc.vector.tensor_tensor(out=u, in0=g1, in1=s_raw, op=AL.mult)
                pr_mn = sm.tile([P, WI], f32, tag="prmn", bufs=1)
                nc.vector.tensor_reduce(out=pr_mn, in_=u.rearrange("p (wi b) -> p wi b", b=S),
                                        axis=mybir.AxisListType.X, op=AL.add)
                u2 = sm.tile([P, FPC], f32, tag="t2", bufs=1)
                nc.vector.tensor_tensor(out=u2, in0=g1, in1=mx, op=AL.mult)
                pr_mx = sm.tile([P, WI], f32, tag="prmx", bufs=1)
                nc.vector.tensor_reduce(out=pr_mx, in_=u2.rearrange("p (wi b) -> p wi b", b=S),
                                        axis=mybir.AxisListType.X, op=AL.max)

                # fold patch rows via transpose: [row, wi] -> [wi, row] -> [wi, hi]
                def fold4(src, op, nm):
                    pt = ps.tile([WI, P], f32, tag="pt", bufs=1)
                    nc.tensor.transpose(pt, src, identf)
                    pte = sm.tile([WI, P], f32, tag=nm + "e", bufs=1)
                    nc.scalar.copy(out=pte, in_=pt)
                    ptv = pte.rearrange("q (hi a) -> q hi a", a=S)
                    fa = sm.tile([WI, HIV, 2], f32, tag=nm + "f", bufs=1)
                    nc.vector.tensor_tensor(out=fa, in0=ptv[:, :, 0:2], in1=ptv[:, :, 2:4], op=op)
                    out = sm.tile([WI, HIV], f32, tag=nm + "o", bufs=1)
                    nc.vector.tensor_tensor(out=out, in0=fa[:, :, 0], in1=fa[:, :, 1], op=op)
                    return out

                mnT = fold4(pr_mn, AL.add, "mn")
                mxT = fold4(pr_mx, AL.max, "mxt")

                # g2 = sigmoid(c2w0*mn/1024 + c2w1*mx + c2b) on [wi, hi]
                tg = sm.tile([WI, HIV], f32, tag="tg", bufs=1)
                nc.vector.tensor_scalar(out=tg, in0=mnT, scalar1=cws[0:WI, 3:4], scalar2=1.0 / (C * S * S),
                                        op0=AL.mult, op1=AL.mult)
                tg2 = sm.tile([WI, HIV], f32, tag="tg2", bufs=1)
                nc.vector.tensor_scalar_mul(out=tg2, in0=mxT, scalar1=cws[0:WI, 4:5])
                nc.vector.tensor_tensor(out=tg, in0=tg, in1=tg2, op=AL.add)
                g2t2 = sm.tile([WI, P], f32, tag="g2t2", bufs=1)
                nc.scalar.activation(out=g2t2.rearrange("q (hi a) -> q hi a", a=S),
                                     in_=tg.unsqueeze(2).broadcast_to([WI, HIV, S]),
                                     func=AF.Sigmoid, bias=cws[0:WI, 5:6], scale=1.0)
                pg = ps.tile([P, WI], f32, tag="pg", bufs=1)
                nc.tensor.transpose(pg, g2t2, identf[0:WI, 0:WI])
                g2d = sm.tile([P, WI], f32, tag="g2d", bufs=1)
                nc.vector.tensor_copy(out=g2d, in_=pg)

                # G = g1 * g2 (bf16, per pixel of this half)
                G = sm.tile([P, FPC], bf16, tag="G", bufs=1)
                nc.vector.tensor_tensor(
                    out=G.rearrange("p (wi b) -> p wi b", b=S),
                    in0=g1.rearrange("p (wi b) -> p wi b", b=S),
                    in1=g2d.unsqueeze(2).broadcast_to([P, WI, S]),
                    op=AL.mult)

                yield
                # ---------- Phase 5: out = p1 * G, store ------------------------
                for p1t, ct0, nct in p1s:
                    ot = big.tile([P, nct * TF], bf16, tag=f"xb{v}", bufs=NT)
                    nc.vector.tensor_tensor(
                        out=ot.rearrange("p (c w) -> p c w", c=nct * CT),
                        in0=p1t.rearrange("p (c w) -> p c w", c=nct * CT),
                        in1=G.unsqueeze(1).broadcast_to([P, nct * CT, FPC]),
                        op=AL.mult)
                    nc.gpsimd.dma_start(out=y_v[v, :, ct0 * CT:(ct0 + nct) * CT, :],
                                        in_=ot.rearrange("p (c w) -> p c w", c=nct * CT))

            gens = [emit_half(v) for v in range(NV)]
            for stage in range(4):        # start, ph1, ph2, ph3 interleaved
                for g_ in gens:
                    next(g_, None)
            for g_ in gens:               # ph4+ph5 per half, in half order
                next(g_, None)
                next(g_, None)

    nc.compile()
    return nc


def _get_nc():
    if "nc" not in _CACHE:
        _CACHE["nc"] = _build()
    return _CACHE["nc"]


def kernel(x, fc_w, fc_b, conv1_w, conv1_b, conv2_w, conv2_b, size, **run_kwargs):
    from concourse.bass_utils import run_bass_kernel_spmd

    assert int(size) == S
    x = np.ascontiguousarray(np.asarray(x, dtype=np.float32))
    fcwT = np.ascontiguousarray(np.asarray(fc_w, dtype=np.float32).T)
    fcb = np.asarray(fc_b, dtype=np.float32)
    cws = np.concatenate([
        np.asarray(conv1_w, np.float32).ravel(), np.asarray(conv1_b, np.float32).ravel(),
        np.asarray(conv2_w, np.float32).ravel(), np.asarray(conv2_b, np.float32).ravel(),
    ]).astype(np.float32)
    assert cws.shape == (6,)
    emat = np.zeros((C, C * S), np.float32)
    for c in range(C):
        emat[c, c * S:(c + 1) * S] = 1.0

    import ml_dtypes
    fcwT = fcwT.astype(ml_dtypes.bfloat16)
    emat = emat.astype(ml_dtypes.bfloat16)

    nc = _get_nc()
    in_maps = [dict(x=x[i], fcwT=fcwT, fcb=fcb, cws=cws, emat=emat) for i in range(B)]
    res = run_bass_kernel_spmd(nc, in_maps, core_ids=list(range(B)), **run_kwargs)
    y = np.stack([res.results[i]["y"] for i in range(B)]).astype(np.float32)
    if run_kwargs:
        _CACHE["last_results"] = res
    return y

